# revision 1
# baseline (speedup 1.0000x reference)
"""Trainium2 Bass kernel for 5x5 patch extraction (ZeroPadding2D + gather).

Full input:  images [8, 128, 128, 32] f32
Full output: [8, 128, 128, 800] f32 where
  out[b, i, j, ki*160 + kj*32 + c] = images_padded[b, i+ki, j+kj, c]
  (zero-padded by 2 on each spatial side).

Sharding: data-parallel over batch; core b handles image b, zero
cross-core communication.

Per-core program: stage the image in SBUF as [128 partitions, 4224]
(row i on partition i, with 2 zero columns of padding on each side of
the 128*32 row). For each ki in 0..4 the (kj, c) block of 160 output
floats at (i, j) is a contiguous sliding window of padded row i+ki-2
starting at element j*32 — so one DMA per ki writes the entire
[i, j, 160] block using an overlapping-window source AP. Output rows
whose source row falls outside the image are zero-filled from a small
zero tile.
"""

from contextlib import ExitStack

import numpy as np

import concourse.bass as bass
import concourse.bacc as bacc
import concourse.mybir as mybir
import concourse.tile as tile
from concourse.bass_utils import run_bass_kernel_spmd

K = 5
H = W = 128
C = 32
B = 8
PAD = (K - 1) // 2  # 2
KC = K * C  # 160
FREE = (W + 2 * PAD) * C  # 4224

_NC_CACHE = {}


def _build_nc(n_chunks: int = 1):
    nc = bacc.Bacc("TRN2", target_bir_lowering=False, debug=False)
    images = nc.dram_tensor(
        "images", [H, W * C], mybir.dt.float32, kind="ExternalInput"
    )
    out = nc.dram_tensor(
        "out", [H, W, K * K * C], mybir.dt.float32, kind="ExternalOutput"
    )

    with ExitStack() as ctx:
        tc = ctx.enter_context(tile.TileContext(nc))
        pool = ctx.enter_context(tc.tile_pool(name="p", bufs=1))
        img = pool.tile([128, FREE], mybir.dt.float32)
        zt = pool.tile([128, KC], mybir.dt.float32)

        nc.vector.memset(img[:, 0 : PAD * C], 0.0)
        nc.vector.memset(img[:, FREE - PAD * C : FREE], 0.0)
        nc.vector.memset(zt[:, :], 0.0)

        nc.sync.dma_start(out=img[:, PAD * C : FREE - PAD * C], in_=images.ap())

        base = img[:, :]
        pstep = base.ap[0][0]  # partition step in elements of the backing tensor

        for ki in range(K):
            di = ki - PAD
            i0 = max(0, -di)
            n_i = H - abs(di)
            p0 = max(0, di)

            per = (n_i + n_chunks - 1) // n_chunks
            off = 0
            while off < n_i:
                cnt = min(per, n_i - off)
                src = bass.AP(
                    base.tensor,
                    base.offset + (p0 + off) * pstep,
                    [[pstep, cnt], [C, W], [1, KC]],
                )
                dst = out[i0 + off : i0 + off + cnt, :, ki * KC : (ki + 1) * KC]
                nc.sync.dma_start(out=dst, in_=src)
                off += cnt

            for i_bad in list(range(0, i0)) + list(range(i0 + n_i, H)):
                nc.sync.dma_start(
                    out=out[i_bad, :, ki * KC : (ki + 1) * KC], in_=zt[:, :]
                )

    nc.compile()
    return nc


def _get_nc(n_chunks: int = 1):
    if n_chunks not in _NC_CACHE:
        _NC_CACHE[n_chunks] = _build_nc(n_chunks)
    return _NC_CACHE[n_chunks]


def run(images: np.ndarray, n_chunks: int = 1, trace: bool = False):
    """Run on 8 cores. Returns (output [8,128,128,800], BassKernelResults)."""
    images = np.ascontiguousarray(np.asarray(images, dtype=np.float32))
    assert images.shape == (B, H, W, C), images.shape
    nc = _get_nc(n_chunks)
    in_maps = [{"images": images[b].reshape(H, W * C)} for b in range(B)]
    res = run_bass_kernel_spmd(nc, in_maps, core_ids=list(range(B)), trace=trace)
    out = np.stack([res.results[b]["out"] for b in range(B)], axis=0)
    return out.reshape(B, H, W, K * K * C), res


def kernel(images: np.ndarray) -> np.ndarray:
    out, _ = run(images)
    return out


# revision 2
# speedup vs baseline: 5.8118x; 5.8118x over previous
"""Trainium2 Bass kernel for 5x5 patch extraction (ZeroPadding2D + gather).

Full input:  images [8, 128, 128, 32] f32
Full output: [8, 128, 128, 800] f32 where
  out[b, i, j, ki*160 + kj*32 + c] = images_padded[b, i+ki, j+kj, c]
  (spatial zero-padding of 2 on each side).

Sharding: data-parallel over batch; core b handles image b; zero
cross-core communication.

Per-core program: stage the image in SBUF as [128 partitions, 4224]
(row i on partition i, 2 zero-columns of padding each side). For each
ki, the (kj, c) block of 160 output floats at (i, j) is a contiguous
sliding window of padded row i+ki-2 starting at element j*32, so one
DMA per ki writes the whole [i, j, 160] block via an
overlapping-window source AP. Row borders are zero-filled from a zero
tile on the second HWDGE queue.

Perf notes (measured on TRN2):
- The HWDGE splits one DMA across n = (largest divisor of the outer
  AP count <= 16) SDMA engines. Odd outer counts (127) pin the whole
  transfer to ONE engine (~20 GB/s); 126 -> 14 engines; 128 -> 16.
  So the 127-row slabs are emitted as 126+1 rows.
- Each DMA gets its own completion semaphore (HWDGE ring-management
  requires <= 1 outstanding DMA per semaphore).
- Concurrent writes to overlapping DRAM ranges from multiple DMAs can
  wedge the device -- all writes here are disjoint.
"""

from contextlib import ExitStack

import numpy as np

import concourse.bass as bass
import concourse.bacc as bacc
import concourse.mybir as mybir
from concourse.bass_utils import run_bass_kernel_spmd

K = 5
H = W = 128
C = 32
B = 8
PAD = (K - 1) // 2  # 2
KC = K * C  # 160
FREE = (W + 2 * PAD) * C  # 4224

_NC_CACHE = {}


def _build_nc():
    nc = bacc.Bacc("TRN2", target_bir_lowering=False, debug=False)
    images = nc.dram_tensor(
        "images", [H, W * C], mybir.dt.float32, kind="ExternalInput"
    )
    out = nc.dram_tensor(
        "out", [H, W, K * K * C], mybir.dt.float32, kind="ExternalOutput"
    )

    with ExitStack() as stack:
        img = stack.enter_context(
            nc.sbuf_tensor("img", [128, FREE], mybir.dt.float32)
        )
        zt = stack.enter_context(nc.sbuf_tensor("zt", [128, KC], mybir.dt.float32))
        s_ms = stack.enter_context(nc.semaphore("s_ms"))
        s_load = stack.enter_context(nc.semaphore("s_load"))
        sA = [stack.enter_context(nc.semaphore(f"sA{i}")) for i in range(8)]
        sZ = [stack.enter_context(nc.semaphore(f"sZ{i}")) for i in range(6)]
        block = stack.enter_context(nc.Block())

        base = img[:, :]
        pstep = base.ap[0][0]  # backing-row size in elements (4224)

        @block.vector
        def _(vector):
            vector.memset(img[:, 0 : PAD * C], 0.0).then_inc(s_ms, 1)
            vector.memset(img[:, FREE - PAD * C : FREE], 0.0).then_inc(s_ms, 1)
            vector.memset(zt[:, :], 0.0).then_inc(s_ms, 1)

        @block.sync
        def _(sync):
            sync.dma_start(
                img[:, PAD * C : FREE - PAD * C], images.ap()
            ).then_inc(s_load, 16)
            sync.wait_ge(s_load, 16)
            sync.wait_ge(s_ms, 2)
            n_dma = 0
            for ki in range(K):
                di = ki - PAD
                i0 = max(0, -di)
                n_i = H - abs(di)
                p0 = max(0, di)
                chunks = (
                    [(0, n_i)] if n_i % 2 == 0 else [(0, n_i - 1), (n_i - 1, 1)]
                )
                for off, cnt in chunks:
                    src = bass.AP(
                        base.tensor,
                        base.offset + (p0 + off) * pstep,
                        [[pstep, cnt], [C, W], [1, KC]],
                    )
                    dst = out[
                        i0 + off : i0 + off + cnt, :, ki * KC : (ki + 1) * KC
                    ]
                    sync.dma_start(dst, src).then_inc(sA[n_dma], 16)
                    n_dma += 1
            for i in range(n_dma):
                sync.wait_ge(sA[i], 16)

        @block.scalar
        def _(scalar):
            scalar.wait_ge(s_ms, 3)
            nz = 0
            for ki in range(K):
                di = ki - PAD
                i0 = max(0, -di)
                n_i = H - abs(di)
                for i_bad in list(range(0, i0)) + list(range(i0 + n_i, H)):
                    scalar.dma_start(
                        out[i_bad, :, ki * KC : (ki + 1) * KC], zt[:, :]
                    ).then_inc(sZ[nz], 16)
                    nz += 1
            for i in range(nz):
                scalar.wait_ge(sZ[i], 16)

    nc.compile()
    return nc


def _get_nc():
    if "nc" not in _NC_CACHE:
        _NC_CACHE["nc"] = _build_nc()
    return _NC_CACHE["nc"]


def run(images: np.ndarray, trace: bool = False, tmpdir=None):
    """Run on 8 cores. Returns (output [8,128,128,800], BassKernelResults)."""
    images = np.ascontiguousarray(np.asarray(images, dtype=np.float32))
    assert images.shape == (B, H, W, C), images.shape
    nc = _get_nc()
    in_maps = [{"images": images[b].reshape(H, W * C)} for b in range(B)]
    res = run_bass_kernel_spmd(
        nc, in_maps, core_ids=list(range(B)), trace=trace, tmpdir=tmpdir
    )
    out = np.stack([res.results[b]["out"] for b in range(B)], axis=0)
    return out.reshape(B, H, W, K * K * C), res


def kernel(images: np.ndarray) -> np.ndarray:
    out, _ = run(images)
    return out
